# revision 22
# baseline (speedup 1.0000x reference)
"""Trainium2 Bass kernel for nn_CrossModalIsomorphismBridge.

Pure data-parallel: batch is sharded across 8 NeuronCores; the tiny Linear
weights are folded into per-core packed inputs on the host.

Math note: for K=2 points on S^3 the Karcher-flow init (the normalized
chordal mean) is already the Frechet mean, so the reference's 5 iterations
are a fixed-point no-op whose only effect is f32 noise through the
ill-conditioned arccos near the clip boundary (verified numerically: the
0-iteration shortcut is *closer* to the f64 result than a faithful f32
5-iteration run, and all eta/det signs are identical).  The kernel therefore
computes barycenter = normalize4(v_hat + t_hat) directly.

Det signs need exact-f32 arithmetic (margins down to ~1e-7); the PE matmul
is pseudo-fp32 (~2e-5), so the 32 det-entry columns are packed exactly on
the host and processed in a separate on-device det pass.
"""

import numpy as np
from contextlib import ExitStack

import concourse.bass as bass
import concourse.tile as tile
from concourse import bacc, mybir
from concourse.bass_utils import run_bass_kernel_spmd

F32 = mybir.dt.float32

B = 262144          # full batch
NCORES = 8
NB = B // NCORES    # 32768 samples per core
P = 128             # partitions
NT = NB // P        # 256 tiles per core
TPB = 16            # tiles per block
NBLK = NT // TPB    # 16 blocks
BS = TPB * P        # 2048 samples per block

# det matrix entries: row i uses quaternion q=8i, column c -> m = 32*i + c
M_D = [32 * i + c for i in range(4) for c in range(4)]

_CACHE = {}


def _build_program():
    nc = bacc.Bacc("TRN2", target_bir_lowering=False, debug=False)

    # packed input: [5, NB + 384]; columns 0:NB = [vis0;vis1;vis2;text;1] data,
    # NB:NB+256 = wcomb (v/t atom planar columns), NB+256:NB+384 = wab (dec cols)
    xt = nc.dram_tensor("xt", [5, NB + 384], F32, kind="ExternalInput")
    # exact det-entry atoms, host-packed: da[p, T, col], col = mod*16 + i*4 + c
    da = nc.dram_tensor("da", [P, NT, 32], F32, kind="ExternalInput")

    bary = nc.dram_tensor("bary", [NB, 128], F32, kind="ExternalOutput")
    dec = nc.dram_tensor("dec", [NB, 2], F32, kind="ExternalOutput")
    eta = nc.dram_tensor("eta", [3, NB], F32, kind="ExternalOutput")

    with tile.TileContext(nc) as tc:
        with ExitStack() as ctx:
            singles = ctx.enter_context(tc.tile_pool(name="singles", bufs=1))
            inp = ctx.enter_context(tc.tile_pool(name="inp", bufs=2))
            work = ctx.enter_context(tc.tile_pool(name="work", bufs=2))
            small = ctx.enter_context(tc.tile_pool(name="small", bufs=2))
            small2 = ctx.enter_context(tc.tile_pool(name="small2", bufs=2))
            outp = ctx.enter_context(tc.tile_pool(name="outp", bufs=2))
            ps = ctx.enter_context(tc.tile_pool(name="ps", bufs=2, space="PSUM"))
            detp = ctx.enter_context(tc.tile_pool(name="detp", bufs=1))

            # weights (one DMA)
            wtab = singles.tile([5, 384], F32)
            nc.sync.dma_start(wtab[:], xt.ap()[:, NB:NB + 384])
            w_vt = wtab[:, 0:256]
            w_ab = wtab[:, 256:384]

            HDET = 4                      # det pass quarters (SBUF budget)
            NTH = NT // HDET

            # ------- det pass (exact f32, Pool-only except 2 ACT ops) -------
            # Emitted piecewise between main blocks so each engine's in-order
            # stream never head-of-line blocks on a cross-engine det dep.
            ETA = detp.tile([P, 3, NT], F32)
            PAIRS = [(0, 1), (0, 2), (0, 3), (1, 2), (1, 3), (2, 3)]
            da_v = da.ap().rearrange("p (h T) col -> h p T col", h=HDET)

            det_state = {}

            def det_stage_a(h):
                # DMA + ACT squares + Pool row-norm sums
                DA = detp.tile([P, NTH, 32], F32, tag="dah")
                nc.sync.dma_start(DA[:], da_v[h])
                dav = DA[:].rearrange("p T (mod i c) -> p mod i c T", mod=2, i=4)
                DN = detp.tile([P, 2, 4, NTH], F32, tag="dn")
                for mod in range(2):
                    DSQ = detp.tile([P, 4, 4, NTH], F32, tag="dsq")
                    dtm = detp.tile([P, 4, NTH], F32, tag="dtm")
                    nc.scalar.activation(
                        DSQ[:], dav[:, mod], mybir.ActivationFunctionType.Square)
                    nc.vector.tensor_add(DN[:, mod], DSQ[:, :, 0], DSQ[:, :, 1])
                    nc.vector.tensor_add(dtm[:], DSQ[:, :, 2], DSQ[:, :, 3])
                    nc.vector.tensor_add(DN[:, mod], DN[:, mod], dtm[:])
                det_state[h] = (DA, dav, DN)

            def det_stage_b(h):
                # ACT sqrt (deps long satisfied by now). ACT Sqrt is only
                # ~7e-6 accurate -> keep the raw sumsq for a Newton refine.
                DA, dav, DN = det_state[h]
                SS = detp.tile([P, 2, 4, NTH], F32, tag="ss")
                nc.scalar.activation(
                    SS[:], DN[:], mybir.ActivationFunctionType.Sqrt)
                det_state[h] = (DA, dav, DN, SS)

            def det_stage_b2(h):
                # DVE: one Newton step s' = (s + x/s)/2 -> ~1ulp sqrt, then
                # rv = 1/(s' + 1e-8) (DVE reciprocal is correctly rounded)
                DA, dav, DN, SS = det_state[h]
                r0 = detp.tile([P, 2, 4, NTH], F32, tag="r0")
                # +1e-30 guards the all-zero rows (text==0): avoids 0*inf=NaN
                nc.vector.tensor_scalar_add(r0[:], SS[:], 1e-30)
                nc.vector.reciprocal(r0[:], r0[:])
                nc.vector.tensor_mul(r0[:], r0[:], DN[:])      # x/s
                nc.vector.tensor_add(r0[:], r0[:], SS[:])      # s + x/s
                nc.vector.tensor_scalar(
                    r0[:], r0[:], 0.5, 1e-8, op0=mybir.AluOpType.mult,
                    op1=mybir.AluOpType.add)                   # s' + eps
                nc.vector.reciprocal(DN[:], r0[:])
                det_state[h] = (DA, dav, DN)

            def det_stage_c(h):
                # Pool: z rows, dets, eta flags
                DA, dav, DN = det_state[h]
                ZD = detp.tile([P, 4, 4, NTH], F32, tag="zd")
                dtm2 = detp.tile([P, 4, NTH], F32, tag="dtm")
                for c in range(4):
                    nc.gpsimd.tensor_mul(ZD[:, :, c], dav[:, 0, :, c], DN[:, 0])
                    nc.gpsimd.tensor_mul(dtm2[:], dav[:, 1, :, c], DN[:, 1])
                    nc.gpsimd.tensor_add(ZD[:, :, c], ZD[:, :, c], dtm2[:])

                PM = detp.tile([P, 6, NTH], F32, tag="pm")
                QM = detp.tile([P, 6, NTH], F32, tag="qm")
                t1 = detp.tile([P, NTH], F32, tag="t1")
                t2 = detp.tile([P, NTH], F32, tag="t2")
                acc = detp.tile([P, NTH], F32, tag="acc")

                def det4(col, rows):
                    for k, (a, b) in enumerate(PAIRS):
                        nc.gpsimd.tensor_mul(t1[:], rows[0][a], rows[1][b])
                        nc.gpsimd.tensor_mul(t2[:], rows[0][b], rows[1][a])
                        nc.gpsimd.tensor_sub(PM[:, k], t1[:], t2[:])
                        nc.gpsimd.tensor_mul(t1[:], rows[2][a], rows[3][b])
                        nc.gpsimd.tensor_mul(t2[:], rows[2][b], rows[3][a])
                        nc.gpsimd.tensor_sub(QM[:, k], t1[:], t2[:])
                    # det = P01*Q23 - P02*Q13 + P03*Q12
                    #     + P12*Q03 - P13*Q02 + P23*Q01
                    nc.gpsimd.tensor_mul(acc[:], PM[:, 0], QM[:, 5])
                    nc.gpsimd.tensor_mul(t1[:], PM[:, 1], QM[:, 4])
                    nc.gpsimd.tensor_sub(acc[:], acc[:], t1[:])
                    nc.gpsimd.tensor_mul(t1[:], PM[:, 2], QM[:, 3])
                    nc.gpsimd.tensor_add(acc[:], acc[:], t1[:])
                    nc.gpsimd.tensor_mul(t1[:], PM[:, 3], QM[:, 2])
                    nc.gpsimd.tensor_add(acc[:], acc[:], t1[:])
                    nc.gpsimd.tensor_mul(t1[:], PM[:, 4], QM[:, 1])
                    nc.gpsimd.tensor_sub(acc[:], acc[:], t1[:])
                    nc.gpsimd.tensor_mul(t1[:], PM[:, 5], QM[:, 0])
                    nc.gpsimd.tensor_add(acc[:], acc[:], t1[:])
                    nc.gpsimd.tensor_scalar(
                        ETA[:, col, h * NTH:(h + 1) * NTH], acc[:],
                        0.0, None, op0=mybir.AluOpType.is_lt,
                    )

                v_rows = [[dav[:, 0, i, c] for c in range(4)] for i in range(4)]
                t_rows = [[dav[:, 1, i, c] for c in range(4)] for i in range(4)]
                z_rows = [[ZD[:, i, c] for c in range(4)] for i in range(4)]
                det4(0, v_rows)
                det4(1, t_rows)
                det4(2, z_rows)

            # emission schedule: stage_a(h) before blocks, stage_b/c spread out
            det_schedule = {}
            for _h in range(HDET):
                det_schedule.setdefault(4 * _h + 0, []).append(
                    (lambda hh: lambda: det_stage_a(hh))(_h))
                det_schedule.setdefault(4 * _h + 1, []).append(
                    (lambda hh: lambda: det_stage_b(hh))(_h))
                det_schedule.setdefault(4 * _h + 3, []).append(
                    (lambda hh: lambda: det_stage_b2(hh))(_h))
                det_schedule.setdefault(4 * _h + 3, []).append(
                    (lambda hh: lambda: det_stage_c(hh))(_h))

            bary_v = bary.ap().rearrange("(blk t p) m -> blk p t m", t=TPB, p=P)
            dec_v = dec.ap().rearrange("(blk t p) j -> blk p t j", t=TPB, p=P)

            for blk in range(NBLK):
                for fn_ in det_schedule.get(blk, []):
                    fn_()
                inT = inp.tile([5, BS], F32, tag="inT")
                nc.sync.dma_start(inT[:], xt.ap()[:, blk * BS:(blk + 1) * BS])

                # AVT: planar atoms [mod, c, t, q]; z overwrites the v planes
                AVT = work.tile([P, 2, 4, TPB, 32], F32, tag="avt")
                ABsb = work.tile([P, TPB, 128], F32, tag="absb")
                HT = TPB // 2
                for h in range(2):
                    ps_vt = ps.tile([P, HT, 256], F32, tag="ps")
                    for th in range(HT):
                        t = h * HT + th
                        st = inT[:, t * P:(t + 1) * P]
                        nc.tensor.matmul(ps_vt[:, th, :], st, w_vt,
                                         start=True, stop=True)
                    # planar copy PSUM -> SBUF: (th, mod, c, q) -> (mod, c, t, q)
                    nc.scalar.activation(
                        AVT[:, :, :, h * HT:(h + 1) * HT, :],
                        ps_vt[:].rearrange(
                            "p th (mod c q) -> p mod c th q", mod=2, c=4),
                        mybir.ActivationFunctionType.Copy)
                    ps_ab = ps.tile([P, HT, 128], F32, tag="ps")
                    for th in range(HT):
                        t = h * HT + th
                        st = inT[:, t * P:(t + 1) * P]
                        nc.tensor.matmul(ps_ab[:, th, :], st, w_ab,
                                         start=True, stop=True)
                    nc.scalar.activation(
                        ABsb[:, h * HT:(h + 1) * HT, :], ps_ab[:],
                        mybir.ActivationFunctionType.Copy)

                av = AVT[:, 0]      # [P, 4, TPB, 32] v atoms (becomes z)
                at = AVT[:, 1]      # [P, 4, TPB, 32] t atoms

                # squares of both modalities in one ACT op
                SQ = work.tile([P, 2, 4, TPB, 32], F32, tag="sq")
                nc.scalar.activation(SQ[:], AVT[:],
                                     mybir.ActivationFunctionType.Square)

                # group sums of squares -> [P, 2, TPB, 32] (both mods at once)
                svt2 = small.tile([P, 2, TPB, 32], F32, tag="svt2")
                tmp2 = small.tile([P, 2, TPB, 32], F32, tag="tmp2")
                nc.vector.tensor_add(svt2[:], SQ[:, :, 0], SQ[:, :, 1])
                nc.vector.tensor_add(tmp2[:], SQ[:, :, 2], SQ[:, :, 3])
                nc.vector.tensor_add(svt2[:], svt2[:], tmp2[:])

                # rnv/rnt = 1/(sqrt(s)+1e-8), fused over both modalities
                # (the reference adds eps only after the norm; the extra eps on
                #  the v side shifts v_hat by <2e-6 relative - far inside the
                #  f32 envelope - and t NEEDS it since t_atoms can be exactly 0)
                rnvt = small2.tile([P, 2, TPB, 32], F32, tag="rnvt")
                nc.scalar.activation(rnvt[:], svt2[:],
                                     mybir.ActivationFunctionType.Sqrt)
                nc.vector.tensor_scalar_add(rnvt[:], rnvt[:], 1e-8)
                nc.vector.reciprocal(rnvt[:], rnvt[:])
                rnv = rnvt[:, 0]
                rnt = rnvt[:, 1]

                # z = v*rnv + t*rnt  (in place over the v planes; rn
                # broadcast over the component dim via stride-0 APs)
                rnv_b = rnvt[:, 0:1].broadcast_to([P, 4, TPB, 32])
                rnt_b = rnvt[:, 1:2].broadcast_to([P, 4, TPB, 32])
                zt4 = work.tile([P, 4, TPB, 32], F32, tag="zt")
                nc.vector.tensor_mul(av[:], av[:], rnv_b)
                nc.vector.tensor_mul(zt4[:], at[:], rnt_b)
                nc.vector.tensor_add(av[:], av[:], zt4[:])

                # rnz = 1/sqrt(sz2)
                SQZ = work.tile([P, 4, TPB, 32], F32, tag="sq")
                nc.scalar.activation(SQZ[:], av[:],
                                     mybir.ActivationFunctionType.Square)
                sz2 = small.tile([P, TPB, 32], F32, tag="sz2")
                tmpz = small.tile([P, TPB, 32], F32, tag="tmp2")
                nc.vector.tensor_add(sz2[:], SQZ[:, 0], SQZ[:, 1])
                nc.vector.tensor_add(tmpz[:], SQZ[:, 2], SQZ[:, 3])
                nc.vector.tensor_add(sz2[:], sz2[:], tmpz[:])
                rnz = small2.tile([P, TPB, 32], F32, tag="rnz")
                nc.scalar.activation(rnz[:], sz2[:],
                                     mybir.ActivationFunctionType.Sqrt)
                nc.vector.reciprocal(rnz[:], rnz[:])

                # barycenter (interleaved m = 4q + c) on Pool, then DMA out
                BARY = outp.tile([P, TPB, 128], F32, tag="bary")
                bview = BARY[:].rearrange("p t (q c) -> p c t q", c=4)
                rnz_b = rnz[:].unsqueeze(1).broadcast_to([P, 4, TPB, 32])
                nc.gpsimd.tensor_mul(bview[:], av[:], rnz_b)
                nc.sync.dma_start(bary_v[blk], BARY[:])

                # decision: dec_j = sum_{mod,q} [a_j|b_j] * [rnz*rnv|rnz*rnt]
                PQ = small2.tile([P, TPB, 2, 32], F32, tag="PQ")
                nc.vector.tensor_mul(PQ[:, :, 0, :], rnz[:], rnv)
                nc.vector.tensor_mul(PQ[:, :, 1, :], rnz[:], rnt)
                DEC = outp.tile([P, TPB, 2], F32, tag="dec")
                ej = small2.tile([P, TPB, 64], F32, tag="ej")
                pq_flat = PQ[:].rearrange("p t m q -> p t (m q)")
                for j in range(2):
                    nc.vector.tensor_mul(
                        ej[:], ABsb[:, :, j * 64:(j + 1) * 64], pq_flat)
                    nc.vector.tensor_reduce(
                        DEC[:, :, j], ej[:], axis=mybir.AxisListType.X,
                        op=mybir.AluOpType.add,
                    )
                nc.sync.dma_start(dec_v[blk], DEC[:])

            eta_v_dram = eta.ap().rearrange("k (T p) -> p k T", p=P)
            nc.sync.dma_start(eta_v_dram, ETA[:])

    nc.compile()
    return nc


def _get_program():
    if "nc" not in _CACHE:
        _CACHE["nc"] = _build_program()
    return _CACHE["nc"]


def _pack_host(vision_ycbcr, text_bytes, Wv, bv, Wt, bt, Wd, bd):
    """Build per-core packed inputs. Returns list of in_maps."""
    f32 = np.float32
    vis = np.asarray(vision_ycbcr, f32)
    txt = np.asarray(text_bytes)
    Wv = np.asarray(Wv, f32); bv = np.asarray(bv, f32)
    Wt = np.asarray(Wt, f32); bt = np.asarray(bt, f32)
    Wd = np.asarray(Wd, f32); bd = np.asarray(bd, f32)

    # wcomb [5, 256]: planar columns j: mod=j//128, c=(j%128)//32, q=j%32, m=4q+c
    wcomb = np.zeros((5, 256), f32)
    q = np.arange(32)
    for c in range(4):
        m = 4 * q + c
        wcomb[0:3, 32 * c + q] = Wv[m, :].T
        wcomb[4, 32 * c + q] = bv[m]
        wcomb[3, 128 + 32 * c + q] = Wt[m, 0]
        wcomb[4, 128 + 32 * c + q] = bt[m]

    # wab [5, 128]: col k = j*64 + mod*32 + q (folded through Wd, in f64)
    Wv64 = Wv.astype(np.float64); Wd64 = Wd.astype(np.float64)
    Wt64 = Wt.astype(np.float64)
    wab = np.zeros((5, 128), np.float64)
    for j in range(2):
        for qq in range(32):
            m = 4 * qq + np.arange(4)
            wab[0:3, j * 64 + qq] = (Wd64[j, m][:, None] * Wv64[m, :]).sum(0)
            wab[4, j * 64 + qq] = (Wd64[j, m] * bv[m].astype(np.float64)).sum()
            wab[3, j * 64 + 32 + qq] = (Wd64[j, m] * Wt64[m, 0]).sum()
            wab[4, j * 64 + 32 + qq] = (Wd64[j, m] * bt[m].astype(np.float64)).sum()
    wab = wab.astype(f32)

    # exact det-entry atoms (f32 host math, matching the reference's f32 linears)
    md = np.array(M_D)
    va_d = vis @ Wv[md, :].T.astype(f32) + bv[md]          # [B, 16]
    ta_d = txt.astype(f32) @ Wt[md, :].T.astype(f32) + bt[md]
    da_full = np.concatenate([va_d, ta_d], axis=1)          # [B, 32] (mod,i,c)

    in_maps = []
    for cidx in range(NCORES):
        sl = slice(cidx * NB, (cidx + 1) * NB)
        xt = np.empty((5, NB + 384), f32)
        xt[0:3, :NB] = vis[sl].T
        xt[3, :NB] = txt[sl, 0].astype(f32)
        xt[4, :NB] = 1.0
        xt[:, NB:NB + 256] = wcomb
        xt[:, NB + 256:NB + 384] = wab
        # da: [P, T, 32] with sample s = 128*T + p
        da = np.ascontiguousarray(
            da_full[sl].reshape(NT, P, 32).transpose(1, 0, 2))
        in_maps.append({"xt": np.ascontiguousarray(xt), "da": da})
    return in_maps


def kernel(vision_ycbcr, text_bytes, Wv, bv, Wt, bt, Wd, bd, _want_trace=False):
    assert vision_ycbcr.shape == (B, 3)
    nc = _get_program()
    in_maps = _pack_host(vision_ycbcr, text_bytes, Wv, bv, Wt, bt, Wd, bd)
    res = run_bass_kernel_spmd(
        nc, in_maps, core_ids=list(range(NCORES)), trace=_want_trace)

    f32 = np.float32
    bary = np.concatenate([res.results[c]["bary"] for c in range(NCORES)], 0)
    dec = np.concatenate([res.results[c]["dec"] for c in range(NCORES)], 0)
    etas = [res.results[c]["eta"] for c in range(NCORES)]  # each [3, NB]
    eta_v = np.concatenate([e[0] for e in etas]).astype(f32)
    eta_t = np.concatenate([e[1] for e in etas]).astype(f32)
    eta_b = np.concatenate([e[2] for e in etas]).astype(f32)

    logic_curvature = np.float32(np.abs(eta_v - eta_t).sum(dtype=np.float64)
                                 * (1.0 / B))
    iso_loss = (np.float32(np.sqrt(np.float32(((eta_v - eta_b) ** 2)
                                              .sum(dtype=np.float64))))
                + np.float32(np.sqrt(np.float32(((eta_t - eta_b) ** 2)
                                                .sum(dtype=np.float64)))))
    out = (bary, eta_v, eta_t, eta_b, np.float32(iso_loss),
           np.float32(logic_curvature), dec)
    if _want_trace:
        return out, res
    return out
